# revision 8
# baseline (speedup 1.0000x reference)
"""Channelwise symmetric Hausdorff distance loss on 8 Trainium2 NeuronCores.

Math (per (batch, channel) pair; x, y are [N, D] point sets):
    d2[n, m] = |x_n|^2 + |y_m|^2 - 2 x_n.y_m
    h = max( max_n min_m d(n,m), max_m min_n d(n,m) )
    answer   = mean over the B*C pairs of h.

Sharding: B*C = 24 pairs, 3 per NeuronCore (data parallel), host gathers.

Per-core device kernel (v2):
  - host-prepped inputs, partition-major so each DMA moves 4 KB contiguous
    per partition (large packets stream ~4x faster per queue than the 1 KB
    packets the old chunked layout produced):
      xtp = (-2 x)^T fp8 [128, KT*N] (stationary side, chunk-major per part.)
      ytp = y^T      fp8 [128, KT*N] (moving side)
      y2bc fp16 [128, N] (y2 broadcast to all partitions), x2s fp32.
  - all input DMAs trigger up-front, spread over 3 engine queues (sync /
    gpsimd / scalar) so trigger serialization (~0.6us each) stops gating
    the DMA engines; SBUF is big enough to hold all 3 pairs at once.
  - ~10 warm-up matmuls on scratch data run while the first DMAs land,
    flipping the PE HAM clock gate from 1.2 to 2.4 GHz before real work.
  - per n-tile (software-pipelined across the whole (pair, n-tile) list):
      1. scalar engine PRE-LOADS the psum tile with y2bc (engine writes to
         PSUM set the has_written bits, so accumulating matmuls add on top;
         verified on HW). Preloads are issued one tile ahead of copy-outs
         so the PE never waits on the scalar engine.
      2. 8 accumulating fp8 DoubleRow matmuls (start=False): psum += -2 x.y
         -> psum = y2 - 2 x.y.  This removes the 16 fold-in matmuls per
         pair the old kernel spent ~20% of PE time on.
      3. scalar engine copies psum out as fp16 d2p [128, N] in SBUF.
      4. DVE (fp16, packed 2x/lane): rowaccs[:, idx] = min_m d2p, and
         colacc = min(colacc, d2p + x2[n]) (per-partition scalar fold).
  - outputs: rowaccs [128, PP*NT] fp16, colacc [128, N] fp16 per pair.
Host finishes in float64: fwd2 = max(x2 + rowaccs), bwd2 =
max_m(min_p colacc), h = sqrt(max(fwd2, bwd2, 0)), mean over 24 pairs.
"""

import numpy as np

B, C, N, D = 8, 3, 1024, 1024
N_CORES = 8
PAIRS = B * C              # 24
PP = PAIRS // N_CORES      # 3 pairs per core
NT = N // 128              # 8 n-tiles (output partition dim)
MBS = 512                  # m block size (one PSUM bank of fp32)
MB = N // MBS              # 2 m-blocks
KT = D // 128              # 8 k-tiles (contraction)
HALF = KT * N // 2         # fp8 bytes per partition per half-tensor DMA
N_WARMUP = 10              # warm-up matmuls (~4us cold -> HAM 8/8)

_NC_CACHE = None


def _legalize_sync(nc):
    """This toolchain's walrus accepts at most ONE sync-wait per instruction;
    Tile emits several (e.g. the tail drain waits on every engine/DMA sem).
    Hoist all but the last wait of each instruction into standalone
    InstEventSemaphore instructions on the same engine, inserted just before
    it — semantically identical (the engine blocks on each in turn)."""
    import concourse.mybir as mybir

    n_split = 0
    for fn in nc.m.functions:
        for bb in fn.blocks:
            new_il = []
            for ins in bb.instructions:
                si = ins.sync_info
                if si is not None and si.on_wait and len(si.on_wait) > 1:
                    waits = list(si.on_wait)
                    for k, w in enumerate(waits[:-1]):
                        ev = mybir.InstEventSemaphore(
                            name=f"{ins.name}-evw{k}",
                            engine=ins.engine,
                            ins=[],
                            outs=[],
                            sync_info=mybir.SyncInfo(on_wait=[w], on_update=[]),
                        )
                        new_il.append(ev)
                        n_split += 1
                    si.on_wait = [waits[-1]]
                new_il.append(ins)
            bb.instructions[:] = new_il
    return n_split


def _build_nc():
    import concourse.bass as bass
    import concourse.mybir as mybir
    import concourse.tile as tile

    f16 = mybir.dt.float16
    f32 = mybir.dt.float32
    f8 = mybir.dt.float8e4
    op_add = mybir.AluOpType.add
    op_min = mybir.AluOpType.min

    nc = bass.Bass("TRN2", target_bir_lowering=True, debug=False)
    xt_d = nc.dram_tensor("xtp", [PP, 128, KT * N], f8, kind="ExternalInput").ap()
    yt_d = nc.dram_tensor("ytp", [PP, 128, KT * N], f8, kind="ExternalInput").ap()
    ybc_d = nc.dram_tensor("y2bc", [PP, 128, N], f16, kind="ExternalInput").ap()
    x2_d = nc.dram_tensor("x2s", [128, PP * NT], f32, kind="ExternalInput").ap()
    row_d = nc.dram_tensor("rowout", [128, PP * NT], f16, kind="ExternalOutput").ap()
    col_d = nc.dram_tensor("colout", [PP, 128, N], f16, kind="ExternalOutput").ap()

    with tile.TileContext(nc) as tc:
        with (
            tc.tile_pool(name="const", bufs=1) as const_pool,
            tc.tile_pool(name="xy", bufs=3) as xy_pool,
            tc.tile_pool(name="bc", bufs=3) as bc_pool,
            tc.tile_pool(name="d2", bufs=3) as d2_pool,
            tc.tile_pool(name="col", bufs=2) as col_pool,
            tc.tile_pool(name="ps", bufs=4, space="PSUM") as ps_pool,
        ):
            ones2 = const_pool.tile([2, 128], f16)
            nc.vector.memset(ones2, 1.0)
            wu_mov = const_pool.tile([2, MBS], f16)
            nc.vector.memset(wu_mov, 1.0)
            x2_sb = const_pool.tile([128, PP * NT], f32)
            rowaccs = const_pool.tile([128, PP * NT], f16)

            # ---- input DMAs: all triggered up-front, 3 trigger queues ----
            xt_sb = [
                xy_pool.tile([128, KT * N], f8, tag="xt", name=f"xt{j}")
                for j in range(PP)
            ]
            yt_sb = [
                xy_pool.tile([128, KT * N], f8, tag="yt", name=f"yt{j}")
                for j in range(PP)
            ]
            ybc_sb = [
                bc_pool.tile([128, N], f16, tag="ybc", name=f"ybc{j}")
                for j in range(PP)
            ]
            trig = [nc.sync, nc.gpsimd, nc.scalar]
            nc.sync.dma_start(out=ybc_sb[0], in_=ybc_d[0])
            nc.sync.dma_start(out=xt_sb[0][:, :HALF], in_=xt_d[0, :, :HALF])
            nc.sync.dma_start(out=yt_sb[0][:, :HALF], in_=yt_d[0, :, :HALF])
            nc.sync.dma_start(out=x2_sb, in_=x2_d)
            nc.sync.dma_start(out=xt_sb[0][:, HALF:], in_=xt_d[0, :, HALF:])
            nc.sync.dma_start(out=yt_sb[0][:, HALF:], in_=yt_d[0, :, HALF:])
            for j in (1, 2):
                eng = trig[j]
                eng.dma_start(out=ybc_sb[j], in_=ybc_d[j])
                eng.dma_start(out=xt_sb[j][:, :HALF], in_=xt_d[j, :, :HALF])
                eng.dma_start(out=yt_sb[j][:, :HALF], in_=yt_d[j, :, :HALF])
                eng.dma_start(out=xt_sb[j][:, HALF:], in_=xt_d[j, :, HALF:])
                eng.dma_start(out=yt_sb[j][:, HALF:], in_=yt_d[j, :, HALF:])

            # ---- PE warm-up: flip HAM to 8/8 while DMAs land ----
            ps_wu = ps_pool.tile([128, MB, MBS], f32, tag="ps")
            for i in range(N_WARMUP):
                nc.tensor.matmul(
                    ps_wu[:, i % MB, :], ones2, wu_mov, start=True, stop=True
                )

            units = [(j, nt) for j in range(PP) for nt in range(NT)]
            colaccs = [None] * PP

            def preload(u):
                ps_u = ps_pool.tile([128, MB, MBS], f32, tag="ps", name=f"ps{u}")
                jv = units[u][0]
                nc.scalar.copy(
                    out=ps_u, in_=ybc_sb[jv].rearrange("p (a m) -> p a m", a=MB)
                )
                return ps_u

            ps_next = preload(0)
            for u, (j, nt) in enumerate(units):
                ps = ps_next
                xt3 = xt_sb[j].rearrange("p (k n) -> p k n", k=KT)
                yt3 = yt_sb[j].rearrange("p (k n) -> p k n", k=KT)
                nsl = slice(nt * 128, (nt + 1) * 128)
                for ki in range(KT // 2):
                    xsl = xt3[:, 2 * ki : 2 * ki + 2, nsl]
                    for mb in range(MB):
                        nc.tensor.matmul(
                            ps[:, mb, :],
                            xsl,
                            yt3[:, 2 * ki : 2 * ki + 2, mb * MBS : (mb + 1) * MBS],
                            start=False,  # accumulate onto the y2bc preload
                            stop=(ki == KT // 2 - 1 and mb == MB - 1),
                            perf_mode=mybir.MatmulPerfMode.DoubleRow,
                        )
                # preload the NEXT tile before this one's copy-out so the
                # scalar engine stays a tile ahead of the PE
                if u + 1 < len(units):
                    ps_next = preload(u + 1)
                # copy-out: d2p = psum (= y2 - 2 x.y) as fp16 in SBUF
                d2p = d2_pool.tile([128, N], f16, tag="d2p")
                nc.scalar.copy(
                    out=d2p.rearrange("p (a m) -> p a m", a=MB), in_=ps
                )
                idx = j * NT + nt
                # rowaccs[:, idx] = min_m(y2[m] - 2 x.y)   (fp16 packed)
                nc.vector.tensor_reduce(
                    out=rowaccs[:, idx : idx + 1],
                    in_=d2p,
                    axis=mybir.AxisListType.X,
                    op=op_min,
                )
                # colacc = min(colacc, d2p + x2[n])        (fp16 packed)
                if nt == 0:
                    colaccs[j] = col_pool.tile(
                        [128, N], f16, tag="colacc", name=f"colacc{j}"
                    )
                    nc.vector.tensor_scalar(
                        out=colaccs[j],
                        in0=d2p,
                        scalar1=x2_sb[:, idx : idx + 1],
                        scalar2=None,
                        op0=op_add,
                    )
                else:
                    nc.vector.scalar_tensor_tensor(
                        out=colaccs[j],
                        in0=d2p,
                        scalar=x2_sb[:, idx : idx + 1],
                        in1=colaccs[j],
                        op0=op_add,
                        op1=op_min,
                    )
                if nt == NT - 1:
                    nc.sync.dma_start(out=col_d[j], in_=colaccs[j])
            nc.sync.dma_start(out=row_d, in_=rowaccs)
    _legalize_sync(nc)
    return nc


def _prep_inputs(x, y):
    import ml_dtypes

    f8np = np.dtype(ml_dtypes.float8_e4m3)
    x32 = np.ascontiguousarray(x, dtype=np.float32).reshape(PAIRS, N, D)
    y32 = np.ascontiguousarray(y, dtype=np.float32).reshape(PAIRS, N, D)

    # xtp[q, p, k*N + n] = -2 x[q, n, k*128 + p]; ytp[q, p, k*N+m] = y[q,m,k*128+p]
    xtp = np.empty((PAIRS, 128, KT * N), f8np)
    ytp = np.empty((PAIRS, 128, KT * N), f8np)
    for q in range(PAIRS):
        xt = (x32[q].T * np.float32(-2.0)).astype(f8np)   # [D, N]
        yt = y32[q].T.astype(f8np)
        xtp[q] = xt.reshape(KT, 128, N).transpose(1, 0, 2).reshape(128, KT * N)
        ytp[q] = yt.reshape(KT, 128, N).transpose(1, 0, 2).reshape(128, KT * N)

    x2 = np.square(x32.astype(np.float64)).sum(-1)  # [PAIRS, N]
    y2 = np.square(y32.astype(np.float64)).sum(-1)
    # x2s[q, p, t] = x2[q, t*128 + p]
    x2s = np.ascontiguousarray(
        x2.reshape(PAIRS, NT, 128).transpose(0, 2, 1).astype(np.float32)
    )
    # y2 broadcast to all 128 partitions, fp16 (d2 is fp16 on device anyway)
    y2bc = np.ascontiguousarray(
        np.broadcast_to(y2.astype(np.float16)[:, None, :], (PAIRS, 128, N))
    )
    return xtp, ytp, x2s, y2bc, x2


def _run(x, y, trace=False):
    global _NC_CACHE
    from concourse.bass_utils import run_bass_kernel_spmd

    xtp, ytp, x2s, y2bc, x2 = _prep_inputs(x, y)

    if _NC_CACHE is None:
        _NC_CACHE = _build_nc()
    nc = _NC_CACHE

    in_maps = []
    for i in range(N_CORES):
        q0 = i * PP
        x2s_core = np.ascontiguousarray(
            x2s[q0 : q0 + PP].transpose(1, 0, 2).reshape(128, PP * NT)
        )
        in_maps.append(
            {
                "xtp": xtp[q0 : q0 + PP],
                "ytp": ytp[q0 : q0 + PP],
                "y2bc": y2bc[q0 : q0 + PP],
                "x2s": x2s_core,
            }
        )

    res = run_bass_kernel_spmd(nc, in_maps, core_ids=list(range(N_CORES)), trace=trace)

    h2 = np.empty(PAIRS, np.float64)
    for i in range(N_CORES):
        r = res.results[i]
        row = r["rowout"].astype(np.float64)  # [128, PP*NT]
        for j in range(PP):
            q = i * PP + j
            # rowaccs[p, j*NT+t] = min_m(y2[m] - 2 x.y)  for n = t*128+p
            rmin = row[:, j * NT : (j + 1) * NT]          # [128, NT]
            x2q = x2[q].reshape(NT, 128).T                # [128, NT]
            fwd2 = (rmin + x2q).max()
            # colacc[p, m] = min over n-tiles of full d2 in fp16
            cmin = r["colout"][j].astype(np.float64).min(0)  # [N]
            bwd2 = cmin.max()
            h2[q] = max(fwd2, bwd2, 0.0)

    ans = np.sqrt(h2).mean()
    return np.array(ans, dtype=np.float32), res


def kernel(input, target):
    out, _ = _run(np.asarray(input), np.asarray(target), trace=False)
    return out
